# revision 36
# baseline (speedup 1.0000x reference)
"""Multi-head GAT conv on 8 trn2 NeuronCores (dst-sharded message passing).

Math restructuring (exact, exploits linearity of matmul):
  alpha_s[n,h] = x[n] @ (W_h @ att_src_h)          (node table, built on device)
  alpha_d[n,h] = x[n] @ (W_h @ att_dst_h)
  w_e[h]   = exp(leaky(alpha_s[src_e] + alpha_d[dst_e]))   (no segment-max:
             logits bounded ~|8| for randn inputs, exp is exact in fp32)
  agg_h[v] = (sum_e w_e[h] * x[src_e]) / (sum_e w_e[h])    (segment sums by dst)
  out[v]   = relu( sum_h agg_h[v] @ (W_h @ W_out_h) + b_eff )

Per core (1/8 of dst nodes): edges sorted by dst, processed in blocks of 128
dst nodes; per block all incoming edges gathered 128/chunk via dma_gather
(x rows 256B bf16, alpha rows 256B fp32; int16 gather indices -> two 25k
table halves).  One-hot edge->dst matrices built by iota==dstcol compare
(bf16, DVE 4x mode), scaled per head by w_e; PE matmuls produce the
weighted feature sums (aggT[F, h*128+dst], bf16 inputs, fp32 PSUM accum)
and transposed denominators (lhsT=wexp[e,4], rhs=mask -> denT[4,dst]).
Reciprocals are partition-broadcast with rank-4 selector matmuls, applied
on the PSUM result, then per-head fp32 output matmuls + bias + relu.
ACT runs ONLY Exp (one batched op per block) to avoid LUT-table reloads.
"""

import numpy as np
import ml_dtypes
from contextlib import ExitStack

import concourse.bass as bass
import concourse.bacc as bacc
import concourse.tile as tile
from concourse import mybir
from concourse._compat import get_trn_type
from concourse.bass_utils import run_bass_kernel_spmd
from concourse.masks import make_identity
from concourse.library_config import mlp

F32 = mybir.dt.float32
BF16 = mybir.dt.bfloat16
I16 = mybir.dt.int16
AT = mybir.ActivationFunctionType
OP = mybir.AluOpType
NPBF = ml_dtypes.bfloat16

N = 50000
F = 128
H = 4
NCORES = 8
NLOC = N // NCORES            # 6250 dst nodes per core
P = 128
NBLK = (NLOC + P - 1) // P    # 49 blocks of 128 dst nodes
HALF = N // 2                 # int16 gather index limit -> two table halves
SLOPE = 0.2
BJ = 8                        # table-build batch (tiles per DMA)
NT_ALL = (N + P - 1) // P     # 391 tiles for the alpha table build
NGRP = (NT_ALL + BJ - 1) // BJ          # 49 build groups
NTPAD = NGRP * BJ * P                   # 50176
NGRP_L = (NBLK + BJ - 1) // BJ          # 7 local build groups
NPAD = NGRP_L * BJ * P                  # 7168 (>= 6272 used rows)

_prog_cache = {}


def _build_program(KA, KB):
    """Bass program shared by all 8 cores (SPMD; per-core data differs)."""
    KT = KA + KB
    nc = bacc.Bacc(get_trn_type() or "TRN2", target_bir_lowering=False)

    xT_d = nc.dram_tensor("xT", [F, NTPAD], F32, kind="ExternalInput")
    xas_d = nc.dram_tensor("xas", [NTPAD, 2 * F], BF16, kind="ExternalInput")
    xlocT_d = nc.dram_tensor("xlocT", [F, NPAD], F32, kind="ExternalInput")
    wsd_d = nc.dram_tensor("wsd", [F, 2 * H], F32, kind="ExternalInput")
    wc_d = nc.dram_tensor("wc", [F, H, F], F32, kind="ExternalInput")
    bias_d = nc.dram_tensor("bias_bc", [P, F], F32, kind="ExternalInput")
    iota_d = nc.dram_tensor("iota", [P, P], BF16, kind="ExternalInput")
    sel_d = nc.dram_tensor("sel4", [H, H, P], F32, kind="ExternalInput")
    idx_d = nc.dram_tensor("idxall", [NBLK, P, (2 * KT) * 8], I16,
                           kind="ExternalInput")
    dstc_d = nc.dram_tensor("dstcol", [NBLK, P, KT], F32, kind="ExternalInput")
    out_d = nc.dram_tensor("out", [NBLK * P, F], F32, kind="ExternalOutput")
    # alpha_dst table built on device (fp32 256B rows); alpha_src lands in
    # the xas table (fp32 bits stored in bf16 cols 128:136)
    ad_d = nc.dram_tensor("ad_tab", [NPAD, 64], F32, kind="Internal")

    with ExitStack() as ctx:
        tc = ctx.enter_context(tile.TileContext(nc))
        consts = ctx.enter_context(tc.tile_pool(name="consts", bufs=1))
        build = ctx.enter_context(tc.tile_pool(name="build", bufs=4))
        psb = ctx.enter_context(tc.tile_pool(name="psb", bufs=2, space="PSUM"))
        meta = ctx.enter_context(tc.tile_pool(name="meta", bufs=3))
        gpool = ctx.enter_context(tc.tile_pool(name="gpool", bufs=4))
        small = ctx.enter_context(tc.tile_pool(name="small", bufs=8))
        mpool = ctx.enter_context(tc.tile_pool(name="mpool", bufs=10))
        wtp = ctx.enter_context(tc.tile_pool(name="wtp", bufs=6))
        npool = ctx.enter_context(tc.tile_pool(name="npool", bufs=2))
        opool = ctx.enter_context(tc.tile_pool(name="opool", bufs=3))
        ps_agg = ctx.enter_context(tc.tile_pool(name="ps_agg", bufs=3, space="PSUM"))
        ps_den = ctx.enter_context(tc.tile_pool(name="ps_den", bufs=1, space="PSUM"))
        ps_rb = ctx.enter_context(tc.tile_pool(name="ps_rb", bufs=1, space="PSUM"))
        ps_out = ctx.enter_context(tc.tile_pool(name="ps_out", bufs=1, space="PSUM"))

        nc.gpsimd.load_library(mlp)

        iota_t = consts.tile([P, P], BF16)
        nc.sync.dma_start(iota_t[:], iota_d[:])
        wsd_t = consts.tile([F, 2 * H], F32)
        nc.sync.dma_start(wsd_t[:], wsd_d[:])
        wc_t = consts.tile([F, H, F], F32)
        nc.sync.dma_start(wc_t[:], wc_d[:])
        bias_t = consts.tile([P, F], F32)
        nc.sync.dma_start(bias_t[:], bias_d[:])
        sel_t = consts.tile([H, H, P], F32)
        nc.sync.dma_start(sel_t[:], sel_d[:])

        # ---- node alpha tables: alpha_sd = x @ wsd from host-transposed x,
        # so each tile is a direct lhsT slice (no PE transposes)
        def alpha_tiles(srcT_dram, ngrp, dst_ap, col0):
            for g in range(ngrp):
                r0 = g * BJ * P
                xt8 = build.tile([F, BJ * P], F32, tag="xt8")
                nc.sync.dma_start(xt8[:], srcT_dram[:, r0:r0 + BJ * P])
                al8 = build.tile([P, BJ, H], F32, tag="al8")
                for t0 in range(0, BJ, 4):
                    al_ps = psb.tile([P, 4, 2 * H], F32, tag="ps_misc")
                    for t in range(4):
                        nc.tensor.matmul(al_ps[:, t, :],
                                         lhsT=xt8[:, (t0 + t) * P:(t0 + t + 1) * P],
                                         rhs=wsd_t[:], start=True, stop=True)
                    if t0 == 0:
                        nc.scalar.activation(al8[:, t0:t0 + 4, :],
                                             al_ps[:, :, col0:col0 + H], AT.Copy)
                    else:
                        nc.vector.tensor_copy(al8[:, t0:t0 + 4, :],
                                              al_ps[:, :, col0:col0 + H])
                nc.sync.dma_start(
                    dst_ap[r0:r0 + BJ * P, :].rearrange(
                        "(j p) f -> p j f", p=P), al8[:])

        # alpha_src -> xas cols 128:136 (fp32 bits in bf16 table)
        alpha_tiles(xT_d, NGRP, xas_d[:, F:F + 8].bitcast(F32), 0)
        alpha_tiles(xlocT_d, NGRP_L, ad_d[:, 0:H], H)

        # ---- main loop over dst blocks
        for b in range(NBLK):
            dstc = meta.tile([P, KT], F32, tag="dstc")
            nc.sync.dma_start(dstc[:], dstc_d[b])
            idx = meta.tile([P, (2 * KT) * 8], I16, tag="idx")
            nc.sync.dma_start(idx[:], idx_d[b])
            iA = idx[:, 0:KA * 8]
            iB = idx[:, KA * 8:KT * 8]
            iD = idx[:, KT * 8:2 * KT * 8]

            xg = gpool.tile([P, KT, 2 * F], BF16, tag="xg")
            adg = gpool.tile([P, KT, 64], F32, tag="adg")
            nc.gpsimd.dma_gather(xg[:, 0:KA, :], xas_d[0:HALF, :], iA,
                                 KA * P, KA * P, 2 * F, single_packet=False)
            nc.gpsimd.dma_gather(xg[:, KA:KT, :], xas_d[HALF:N, :], iB,
                                 KB * P, KB * P, 2 * F, single_packet=False)
            nc.gpsimd.dma_gather(adg[:, :, :], ad_d[:, :], iD,
                                 KT * P, KT * P, 64, single_packet=False)
            asg = xg[:, :, F:F + 8].bitcast(F32)

            # batched edge weights: w = exp(leaky(alpha_s + alpha_d))
            lg = small.tile([P, KT, H], F32, tag="lg")
            nc.vector.tensor_tensor(lg[:], asg, adg[:, :, 0:H],
                                    op=OP.add)
            lgs = small.tile([P, KT, H], F32, tag="lgs")
            nc.vector.tensor_scalar(lgs[:], lg[:], SLOPE, None, op0=OP.mult)
            lgm = small.tile([P, KT, H], F32, tag="lgm")
            nc.vector.tensor_tensor(lgm[:], lg[:], lgs[:], op=OP.max)
            wexp = small.tile([P, KT, H], F32, tag="wexp")
            nc.scalar.activation(wexp[:], lgm[:], AT.Exp)
            wexp_b = small.tile([P, KT, H], BF16, tag="wexp_b")
            nc.vector.tensor_copy(wexp_b[:], wexp[:])

            aggT = ps_agg.tile([P, H, P], F32, tag="aggT")
            denT = ps_den.tile([H, P], F32, tag="denT")

            for c in range(KT):
                M = mpool.tile([P, P], BF16, tag="M")
                nc.vector.tensor_scalar(M[:], iota_t[:], dstc[:, c:c + 1],
                                        None, op0=OP.is_equal)
                WT = wtp.tile([P, H, P], BF16, tag="WT")
                for h in range(3):
                    nc.vector.tensor_scalar(WT[:, h, :], M[:],
                                            wexp[:, c, h:h + 1],
                                            None, op0=OP.mult)
                nc.scalar.activation(WT[:, 3, :], M[:], AT.Copy,
                                     scale=wexp[:, c, 3:4])
                nc.tensor.matmul(aggT[:], lhsT=xg[:, c, 0:F], rhs=WT[:],
                                 start=(c == 0), stop=(c == KT - 1))
                nc.tensor.matmul(denT[:], lhsT=wexp_b[:, c, :], rhs=M[:],
                                 start=(c == 0), stop=(c == KT - 1))

            # ---- normalize + output head mix
            rt = small.tile([H, P], F32, tag="rt")
            nc.vector.reciprocal(rt[:], denT[:])
            rb_ps = ps_rb.tile([P, H, P], F32, tag="rb_ps")
            for h in range(H):
                nc.tensor.matmul(rb_ps[:, h, :], lhsT=sel_t[:, h, :],
                                 rhs=rt[:], start=True, stop=True)
            rb = npool.tile([P, H, P], F32, tag="rb")
            nc.vector.tensor_copy(rb[:], rb_ps[:])
            nAT = npool.tile([P, H, P], F32, tag="nAT")
            nc.vector.tensor_tensor(nAT[:], aggT[:], rb[:], op=OP.mult)

            o_ps = ps_out.tile([P, F], F32, tag="o_ps")
            for h in range(H):
                nc.tensor.matmul(o_ps[:], lhsT=nAT[:, h, :], rhs=wc_t[:, h, :],
                                 start=(h == 0), stop=(h == H - 1))
            ob = opool.tile([P, F], F32, tag="ob")
            nc.vector.tensor_tensor(ob[:], o_ps[:], bias_t[:], op=OP.add)
            ob2 = opool.tile([P, F], F32, tag="ob2")
            nc.vector.tensor_scalar(ob2[:], ob[:], 0.0, None, op0=OP.max)
            nc.sync.dma_start(out_d[b * P:(b + 1) * P, :], ob2[:])

    nc.compile()
    return nc


def _pack_idx(arr):
    """[..., n] int -> [..., 128, n//16] int16 wrapped in 16 partitions,
    replicated 8x down the partition dim (Q7 idx layout)."""
    lead = arr.shape[:-1]
    n = arr.shape[-1]
    w = arr.reshape(*lead, n // 16, 16)
    w = np.swapaxes(w, -1, -2)                     # [..., 16, n//16]
    out = np.tile(w, (1,) * len(lead) + (8, 1))    # [..., 128, n//16]
    return np.ascontiguousarray(out.astype(np.int16))


def _host_prep(x, edge_index):
    src = np.concatenate([np.asarray(edge_index[0]), np.arange(N)]).astype(np.int64)
    dst = np.concatenate([np.asarray(edge_index[1]), np.arange(N)]).astype(np.int64)
    order = np.argsort(dst, kind="stable")
    src = src[order]
    dst = dst[order]

    # per (core, block): A = src<HALF edge list, B = rest
    lists = []
    KA = KB = 1
    core_bounds = np.searchsorted(dst, np.arange(NCORES + 1) * NLOC)
    for c in range(NCORES):
        s0, s1 = core_bounds[c], core_bounds[c + 1]
        csrc = src[s0:s1]
        cdst = dst[s0:s1] - c * NLOC
        blk_bounds = np.searchsorted(cdst, np.arange(NBLK + 1) * P)
        per_core = []
        for b in range(NBLK):
            e0, e1 = blk_bounds[b], blk_bounds[b + 1]
            bs = csrc[e0:e1]
            bd = cdst[e0:e1]
            isA = bs < HALF
            eA = (bs[isA], bd[isA])
            eB = (bs[~isA] - HALF, bd[~isA])
            KA = max(KA, (len(eA[0]) + P - 1) // P)
            KB = max(KB, (len(eB[0]) + P - 1) // P)
            per_core.append((eA, eB))
        lists.append(per_core)
    KT = KA + KB

    idxA = np.zeros((NCORES, NBLK, KA * P), np.int64)
    idxB = np.zeros((NCORES, NBLK, KB * P), np.int64)
    idxD = np.zeros((NCORES, NBLK, KT * P), np.int64)
    dstc = np.full((NCORES, NBLK, P, KT), -1.0, np.float32)
    for c in range(NCORES):
        for b in range(NBLK):
            (sA, dA), (sB, dB) = lists[c][b]
            nA, nB = len(sA), len(sB)
            idxA[c, b, :nA] = sA
            idxB[c, b, :nB] = sB
            idxD[c, b, :nA] = dA                      # local dst (0..NLOC)
            idxD[c, b, KA * P:KA * P + nB] = dB
            i = np.arange(nA)
            dstc[c, b, i % P, i // P] = (dA - b * P).astype(np.float32)
            j = np.arange(nB)
            dstc[c, b, j % P, KA + j // P] = (dB - b * P).astype(np.float32)
    idx_all = np.concatenate(
        [_pack_idx(idxA), _pack_idx(idxB), _pack_idx(idxD)], axis=-1)
    return KA, KB, idx_all, dstc


def _make_in_maps(x, ins, KA, KB, idx_all, dstc):
    W = np.asarray(ins["W"], np.float32)
    att_src = np.asarray(ins["att_src"], np.float32)
    att_dst = np.asarray(ins["att_dst"], np.float32)
    bias = np.asarray(ins["bias"], np.float32)
    W_out = np.asarray(ins["W_out"], np.float32)
    b_out = np.asarray(ins["b_out"], np.float32)
    xpad = np.zeros((NTPAD, F), np.float32)
    xpad[:N] = x
    xT = np.ascontiguousarray(xpad.T)
    xas = np.zeros((NTPAD, 2 * F), NPBF)
    xas[:, :F] = xpad.astype(NPBF)

    w_s = np.einsum("hio,ho->ih", W, att_src)          # [F, H]
    w_d = np.einsum("hio,ho->ih", W, att_dst)
    wsd = np.ascontiguousarray(np.concatenate([w_s, w_d], 1))      # [F, 2H]
    Wo = W_out.reshape(H, F, F)
    wc = np.ascontiguousarray(np.einsum("hio,hof->ihf", W, Wo))    # [F, H, F]
    b_eff = bias.reshape(H * F) @ W_out + b_out                    # [F]
    bias_bc = np.ascontiguousarray(np.tile(b_eff[None, :], (P, 1)))
    iota = np.ascontiguousarray(
        np.tile(np.arange(P, dtype=np.float32)[None, :], (P, 1))).astype(NPBF)
    sel4 = np.zeros((H, H, P), np.float32)
    for h in range(H):
        sel4[h, h, :] = 1.0

    in_maps = []
    for c in range(NCORES):
        xloc = np.zeros((NPAD, F), np.float32)
        xloc[:NLOC] = x[c * NLOC:(c + 1) * NLOC]
        in_maps.append({
            "xT": xT, "xas": xas, "xlocT": np.ascontiguousarray(xloc.T), "wsd": wsd, "wc": wc,
            "bias_bc": bias_bc, "iota": iota, "sel4": sel4,
            "idxall": idx_all[c],
            "dstcol": np.ascontiguousarray(dstc[c]),
        })
    return in_maps


def kernel(x, edge_index, W, att_src, att_dst, bias, W_out, b_out):
    x = np.asarray(x, np.float32)
    ins = dict(W=W, att_src=att_src, att_dst=att_dst, bias=bias,
               W_out=W_out, b_out=b_out)

    KA, KB, idx_all, dstc = _host_prep(x, edge_index)
    in_maps = _make_in_maps(x, ins, KA, KB, idx_all, dstc)

    key = (KA, KB)
    if key not in _prog_cache:
        _prog_cache[key] = _build_program(KA, KB)
    nc = _prog_cache[key]

    res = run_bass_kernel_spmd(nc, in_maps, core_ids=list(range(NCORES)))
    out = np.concatenate(
        [res.results[c]["out"][:NLOC] for c in range(NCORES)], 0)
    return out.astype(np.float32)


# revision 37
# speedup vs baseline: 1.0025x; 1.0025x over previous
"""Multi-head GAT conv on 8 trn2 NeuronCores (dst-sharded message passing).

Math restructuring (exact, exploits linearity of matmul):
  alpha_s[n,h] = x[n] @ (W_h @ att_src_h)          (node table, built on device)
  alpha_d[n,h] = x[n] @ (W_h @ att_dst_h)
  w_e[h]   = exp(leaky(alpha_s[src_e] + alpha_d[dst_e]))   (no segment-max:
             logits bounded ~|8| for randn inputs, exp is exact in fp32)
  agg_h[v] = (sum_e w_e[h] * x[src_e]) / (sum_e w_e[h])    (segment sums by dst)
  out[v]   = relu( sum_h agg_h[v] @ (W_h @ W_out_h) + b_eff )

Per core (1/8 of dst nodes): edges sorted by dst, processed in blocks of 128
dst nodes; per block all incoming edges gathered 128/chunk via dma_gather
(x rows 256B bf16, alpha rows 256B fp32; int16 gather indices -> two 25k
table halves).  One-hot edge->dst matrices built by iota==dstcol compare
(bf16, DVE 4x mode), scaled per head by w_e; PE matmuls produce the
weighted feature sums (aggT[F, h*128+dst], bf16 inputs, fp32 PSUM accum)
and transposed denominators (lhsT=wexp[e,4], rhs=mask -> denT[4,dst]).
Reciprocals are partition-broadcast with rank-4 selector matmuls, applied
on the PSUM result, then per-head fp32 output matmuls + bias + relu.
ACT runs ONLY Exp (one batched op per block) to avoid LUT-table reloads.
"""

import numpy as np
import ml_dtypes
from contextlib import ExitStack

import concourse.bass as bass
import concourse.bacc as bacc
import concourse.tile as tile
from concourse import mybir
from concourse._compat import get_trn_type
from concourse.bass_utils import run_bass_kernel_spmd
from concourse.masks import make_identity
from concourse.library_config import mlp

F32 = mybir.dt.float32
BF16 = mybir.dt.bfloat16
I16 = mybir.dt.int16
AT = mybir.ActivationFunctionType
OP = mybir.AluOpType
NPBF = ml_dtypes.bfloat16

N = 50000
F = 128
H = 4
NCORES = 8
NLOC = N // NCORES            # 6250 dst nodes per core
P = 128
NBLK = (NLOC + P - 1) // P    # 49 blocks of 128 dst nodes
HALF = N // 2                 # int16 gather index limit -> two table halves
SLOPE = 0.2
BJ = 8                        # table-build batch (tiles per DMA)
NT_ALL = (N + P - 1) // P     # 391 tiles for the alpha table build
NGRP = (NT_ALL + BJ - 1) // BJ          # 49 build groups
NTPAD = NGRP * BJ * P                   # 50176
NGRP_L = (NBLK + BJ - 1) // BJ          # 7 local build groups
NPAD = NGRP_L * BJ * P                  # 7168 (>= 6272 used rows)

_prog_cache = {}


def _build_program(KA, KB):
    """Bass program shared by all 8 cores (SPMD; per-core data differs)."""
    KT = KA + KB
    nc = bacc.Bacc(get_trn_type() or "TRN2", target_bir_lowering=False)

    xT_d = nc.dram_tensor("xT", [F, NTPAD], F32, kind="ExternalInput")
    xas_d = nc.dram_tensor("xas", [NTPAD, 2 * F], BF16, kind="ExternalInput")
    xlocT_d = nc.dram_tensor("xlocT", [F, NPAD], F32, kind="ExternalInput")
    wsd_d = nc.dram_tensor("wsd", [F, 2 * H], F32, kind="ExternalInput")
    wc_d = nc.dram_tensor("wc", [F, H, F], F32, kind="ExternalInput")
    bias_d = nc.dram_tensor("bias_bc", [P, F], F32, kind="ExternalInput")
    iota_d = nc.dram_tensor("iota", [P, P], BF16, kind="ExternalInput")
    sel_d = nc.dram_tensor("sel4", [H, H, P], F32, kind="ExternalInput")
    idx_d = nc.dram_tensor("idxall", [NBLK, P, (2 * KT) * 8], I16,
                           kind="ExternalInput")
    dstc_d = nc.dram_tensor("dstcol", [NBLK, P, KT], F32, kind="ExternalInput")
    out_d = nc.dram_tensor("out", [NBLK * P, F], F32, kind="ExternalOutput")
    # alpha_dst table built on device (fp32 256B rows); alpha_src lands in
    # the xas table (fp32 bits stored in bf16 cols 128:136)
    ad_d = nc.dram_tensor("ad_tab", [NPAD, 64], F32, kind="Internal")

    with ExitStack() as ctx:
        tc = ctx.enter_context(tile.TileContext(nc))
        consts = ctx.enter_context(tc.tile_pool(name="consts", bufs=1))
        build = ctx.enter_context(tc.tile_pool(name="build", bufs=6))
        psb = ctx.enter_context(tc.tile_pool(name="psb", bufs=2, space="PSUM"))
        meta = ctx.enter_context(tc.tile_pool(name="meta", bufs=4))
        gpool = ctx.enter_context(tc.tile_pool(name="gpool", bufs=5))
        small = ctx.enter_context(tc.tile_pool(name="small", bufs=10))
        mpool = ctx.enter_context(tc.tile_pool(name="mpool", bufs=14))
        wtp = ctx.enter_context(tc.tile_pool(name="wtp", bufs=8))
        npool = ctx.enter_context(tc.tile_pool(name="npool", bufs=2))
        opool = ctx.enter_context(tc.tile_pool(name="opool", bufs=3))
        ps_agg = ctx.enter_context(tc.tile_pool(name="ps_agg", bufs=3, space="PSUM"))
        ps_den = ctx.enter_context(tc.tile_pool(name="ps_den", bufs=1, space="PSUM"))
        ps_rb = ctx.enter_context(tc.tile_pool(name="ps_rb", bufs=1, space="PSUM"))
        ps_out = ctx.enter_context(tc.tile_pool(name="ps_out", bufs=1, space="PSUM"))

        nc.gpsimd.load_library(mlp)

        iota_t = consts.tile([P, P], BF16)
        nc.sync.dma_start(iota_t[:], iota_d[:])
        wsd_t = consts.tile([F, 2 * H], F32)
        nc.sync.dma_start(wsd_t[:], wsd_d[:])
        wc_t = consts.tile([F, H, F], F32)
        nc.sync.dma_start(wc_t[:], wc_d[:])
        bias_t = consts.tile([P, F], F32)
        nc.sync.dma_start(bias_t[:], bias_d[:])
        sel_t = consts.tile([H, H, P], F32)
        nc.sync.dma_start(sel_t[:], sel_d[:])

        # ---- node alpha tables: alpha_sd = x @ wsd from host-transposed x,
        # so each tile is a direct lhsT slice (no PE transposes)
        def alpha_tiles(srcT_dram, ngrp, dst_ap, col0):
            for g in range(ngrp):
                r0 = g * BJ * P
                xt8 = build.tile([F, BJ * P], F32, tag="xt8")
                nc.sync.dma_start(xt8[:], srcT_dram[:, r0:r0 + BJ * P])
                al8 = build.tile([P, BJ, H], F32, tag="al8")
                for t0 in range(0, BJ, 4):
                    al_ps = psb.tile([P, 4, 2 * H], F32, tag="ps_misc")
                    for t in range(4):
                        nc.tensor.matmul(al_ps[:, t, :],
                                         lhsT=xt8[:, (t0 + t) * P:(t0 + t + 1) * P],
                                         rhs=wsd_t[:], start=True, stop=True)
                    if t0 == 0:
                        nc.scalar.activation(al8[:, t0:t0 + 4, :],
                                             al_ps[:, :, col0:col0 + H], AT.Copy)
                    else:
                        nc.vector.tensor_copy(al8[:, t0:t0 + 4, :],
                                              al_ps[:, :, col0:col0 + H])
                nc.sync.dma_start(
                    dst_ap[r0:r0 + BJ * P, :].rearrange(
                        "(j p) f -> p j f", p=P), al8[:])

        # alpha_src -> xas cols 128:136 (fp32 bits in bf16 table)
        alpha_tiles(xT_d, NGRP, xas_d[:, F:F + 8].bitcast(F32), 0)
        alpha_tiles(xlocT_d, NGRP_L, ad_d[:, 0:H], H)

        # ---- main loop over dst blocks
        for b in range(NBLK):
            dstc = meta.tile([P, KT], F32, tag="dstc")
            nc.sync.dma_start(dstc[:], dstc_d[b])
            idx = meta.tile([P, (2 * KT) * 8], I16, tag="idx")
            nc.sync.dma_start(idx[:], idx_d[b])
            iA = idx[:, 0:KA * 8]
            iB = idx[:, KA * 8:KT * 8]
            iD = idx[:, KT * 8:2 * KT * 8]

            xg = gpool.tile([P, KT, 2 * F], BF16, tag="xg")
            adg = gpool.tile([P, KT, 64], F32, tag="adg")
            nc.gpsimd.dma_gather(xg[:, 0:KA, :], xas_d[0:HALF, :], iA,
                                 KA * P, KA * P, 2 * F, single_packet=False)
            nc.gpsimd.dma_gather(xg[:, KA:KT, :], xas_d[HALF:N, :], iB,
                                 KB * P, KB * P, 2 * F, single_packet=False)
            nc.gpsimd.dma_gather(adg[:, :, :], ad_d[:, :], iD,
                                 KT * P, KT * P, 64, single_packet=False)
            asg = xg[:, :, F:F + 8].bitcast(F32)

            # batched edge weights: w = exp(leaky(alpha_s + alpha_d))
            lg = small.tile([P, KT, H], F32, tag="lg")
            nc.vector.tensor_tensor(lg[:], asg, adg[:, :, 0:H],
                                    op=OP.add)
            lgs = small.tile([P, KT, H], F32, tag="lgs")
            nc.vector.tensor_scalar(lgs[:], lg[:], SLOPE, None, op0=OP.mult)
            lgm = small.tile([P, KT, H], F32, tag="lgm")
            nc.vector.tensor_tensor(lgm[:], lg[:], lgs[:], op=OP.max)
            wexp = small.tile([P, KT, H], F32, tag="wexp")
            nc.scalar.activation(wexp[:], lgm[:], AT.Exp)
            wexp_b = small.tile([P, KT, H], BF16, tag="wexp_b")
            nc.vector.tensor_copy(wexp_b[:], wexp[:])

            aggT = ps_agg.tile([P, H, P], F32, tag="aggT")
            denT = ps_den.tile([H, P], F32, tag="denT")

            for c in range(KT):
                M = mpool.tile([P, P], BF16, tag="M")
                nc.vector.tensor_scalar(M[:], iota_t[:], dstc[:, c:c + 1],
                                        None, op0=OP.is_equal)
                WT = wtp.tile([P, H, P], BF16, tag="WT")
                for h in range(3):
                    nc.vector.tensor_scalar(WT[:, h, :], M[:],
                                            wexp[:, c, h:h + 1],
                                            None, op0=OP.mult)
                nc.scalar.activation(WT[:, 3, :], M[:], AT.Copy,
                                     scale=wexp[:, c, 3:4])
                nc.tensor.matmul(aggT[:], lhsT=xg[:, c, 0:F], rhs=WT[:],
                                 start=(c == 0), stop=(c == KT - 1))
                nc.tensor.matmul(denT[:], lhsT=wexp_b[:, c, :], rhs=M[:],
                                 start=(c == 0), stop=(c == KT - 1))

            # ---- normalize + output head mix
            rt = small.tile([H, P], F32, tag="rt")
            nc.vector.reciprocal(rt[:], denT[:])
            rb_ps = ps_rb.tile([P, H, P], F32, tag="rb_ps")
            for h in range(H):
                nc.tensor.matmul(rb_ps[:, h, :], lhsT=sel_t[:, h, :],
                                 rhs=rt[:], start=True, stop=True)
            rb = npool.tile([P, H, P], F32, tag="rb")
            nc.vector.tensor_copy(rb[:], rb_ps[:])
            nAT = npool.tile([P, H, P], F32, tag="nAT")
            nc.vector.tensor_tensor(nAT[:], aggT[:], rb[:], op=OP.mult)

            o_ps = ps_out.tile([P, F], F32, tag="o_ps")
            for h in range(H):
                nc.tensor.matmul(o_ps[:], lhsT=nAT[:, h, :], rhs=wc_t[:, h, :],
                                 start=(h == 0), stop=(h == H - 1))
            ob = opool.tile([P, F], F32, tag="ob")
            nc.vector.tensor_tensor(ob[:], o_ps[:], bias_t[:], op=OP.add)
            ob2 = opool.tile([P, F], F32, tag="ob2")
            nc.vector.tensor_scalar(ob2[:], ob[:], 0.0, None, op0=OP.max)
            nc.sync.dma_start(out_d[b * P:(b + 1) * P, :], ob2[:])

    nc.compile()
    return nc


def _pack_idx(arr):
    """[..., n] int -> [..., 128, n//16] int16 wrapped in 16 partitions,
    replicated 8x down the partition dim (Q7 idx layout)."""
    lead = arr.shape[:-1]
    n = arr.shape[-1]
    w = arr.reshape(*lead, n // 16, 16)
    w = np.swapaxes(w, -1, -2)                     # [..., 16, n//16]
    out = np.tile(w, (1,) * len(lead) + (8, 1))    # [..., 128, n//16]
    return np.ascontiguousarray(out.astype(np.int16))


def _host_prep(x, edge_index):
    src = np.concatenate([np.asarray(edge_index[0]), np.arange(N)]).astype(np.int64)
    dst = np.concatenate([np.asarray(edge_index[1]), np.arange(N)]).astype(np.int64)
    order = np.argsort(dst, kind="stable")
    src = src[order]
    dst = dst[order]

    # per (core, block): A = src<HALF edge list, B = rest
    lists = []
    KA = KB = 1
    core_bounds = np.searchsorted(dst, np.arange(NCORES + 1) * NLOC)
    for c in range(NCORES):
        s0, s1 = core_bounds[c], core_bounds[c + 1]
        csrc = src[s0:s1]
        cdst = dst[s0:s1] - c * NLOC
        blk_bounds = np.searchsorted(cdst, np.arange(NBLK + 1) * P)
        per_core = []
        for b in range(NBLK):
            e0, e1 = blk_bounds[b], blk_bounds[b + 1]
            bs = csrc[e0:e1]
            bd = cdst[e0:e1]
            isA = bs < HALF
            eA = (bs[isA], bd[isA])
            eB = (bs[~isA] - HALF, bd[~isA])
            KA = max(KA, (len(eA[0]) + P - 1) // P)
            KB = max(KB, (len(eB[0]) + P - 1) // P)
            per_core.append((eA, eB))
        lists.append(per_core)
    KT = KA + KB

    idxA = np.zeros((NCORES, NBLK, KA * P), np.int64)
    idxB = np.zeros((NCORES, NBLK, KB * P), np.int64)
    idxD = np.zeros((NCORES, NBLK, KT * P), np.int64)
    dstc = np.full((NCORES, NBLK, P, KT), -1.0, np.float32)
    for c in range(NCORES):
        for b in range(NBLK):
            (sA, dA), (sB, dB) = lists[c][b]
            nA, nB = len(sA), len(sB)
            idxA[c, b, :nA] = sA
            idxB[c, b, :nB] = sB
            idxD[c, b, :nA] = dA                      # local dst (0..NLOC)
            idxD[c, b, KA * P:KA * P + nB] = dB
            i = np.arange(nA)
            dstc[c, b, i % P, i // P] = (dA - b * P).astype(np.float32)
            j = np.arange(nB)
            dstc[c, b, j % P, KA + j // P] = (dB - b * P).astype(np.float32)
    idx_all = np.concatenate(
        [_pack_idx(idxA), _pack_idx(idxB), _pack_idx(idxD)], axis=-1)
    return KA, KB, idx_all, dstc


def _make_in_maps(x, ins, KA, KB, idx_all, dstc):
    W = np.asarray(ins["W"], np.float32)
    att_src = np.asarray(ins["att_src"], np.float32)
    att_dst = np.asarray(ins["att_dst"], np.float32)
    bias = np.asarray(ins["bias"], np.float32)
    W_out = np.asarray(ins["W_out"], np.float32)
    b_out = np.asarray(ins["b_out"], np.float32)
    xpad = np.zeros((NTPAD, F), np.float32)
    xpad[:N] = x
    xT = np.ascontiguousarray(xpad.T)
    xas = np.zeros((NTPAD, 2 * F), NPBF)
    xas[:, :F] = xpad.astype(NPBF)

    w_s = np.einsum("hio,ho->ih", W, att_src)          # [F, H]
    w_d = np.einsum("hio,ho->ih", W, att_dst)
    wsd = np.ascontiguousarray(np.concatenate([w_s, w_d], 1))      # [F, 2H]
    Wo = W_out.reshape(H, F, F)
    wc = np.ascontiguousarray(np.einsum("hio,hof->ihf", W, Wo))    # [F, H, F]
    b_eff = bias.reshape(H * F) @ W_out + b_out                    # [F]
    bias_bc = np.ascontiguousarray(np.tile(b_eff[None, :], (P, 1)))
    iota = np.ascontiguousarray(
        np.tile(np.arange(P, dtype=np.float32)[None, :], (P, 1))).astype(NPBF)
    sel4 = np.zeros((H, H, P), np.float32)
    for h in range(H):
        sel4[h, h, :] = 1.0

    in_maps = []
    for c in range(NCORES):
        xloc = np.zeros((NPAD, F), np.float32)
        xloc[:NLOC] = x[c * NLOC:(c + 1) * NLOC]
        in_maps.append({
            "xT": xT, "xas": xas, "xlocT": np.ascontiguousarray(xloc.T), "wsd": wsd, "wc": wc,
            "bias_bc": bias_bc, "iota": iota, "sel4": sel4,
            "idxall": idx_all[c],
            "dstcol": np.ascontiguousarray(dstc[c]),
        })
    return in_maps


def kernel(x, edge_index, W, att_src, att_dst, bias, W_out, b_out):
    x = np.asarray(x, np.float32)
    ins = dict(W=W, att_src=att_src, att_dst=att_dst, bias=bias,
               W_out=W_out, b_out=b_out)

    KA, KB, idx_all, dstc = _host_prep(x, edge_index)
    in_maps = _make_in_maps(x, ins, KA, KB, idx_all, dstc)

    key = (KA, KB)
    if key not in _prog_cache:
        _prog_cache[key] = _build_program(KA, KB)
    nc = _prog_cache[key]

    res = run_bass_kernel_spmd(nc, in_maps, core_ids=list(range(NCORES)))
    out = np.concatenate(
        [res.results[c]["out"][:NLOC] for c in range(NCORES)], 0)
    return out.astype(np.float32)
